# revision 1
# baseline (speedup 1.0000x reference)
"""Trainium2 Bass kernel for nn_LoRA_QKVlinear (VeRA-style LoRA on K/V of a QKV linear).

Reference computation (fp32):
    delta_k = diag(vera_b[k]) @ vera_B @ diag(vera_d[k]) @ vera_A   for k in {K, V}
    W_eff   = base_weight + concat([0, delta_K, delta_V], axis=0)   # (3072, 1024)
    y       = x @ W_eff.T + base_bias                               # (4, 4096, 3072)

Sharding: data-parallel over tokens (B*S = 16384 -> 2048 per core).  Each of the
8 cores gets the full (replicated) weights + vera tensors and computes the full
3072 output features for its token slice.  No collectives; host concatenates.

Host does layout-only prep (slice/transpose): x -> xT per-core shard [1024, 2048],
base_weight -> W.T [1024, 3072], vera_B -> B.T [256, 1024].  All arithmetic
(vera scaling, delta matmuls, the big matmul, bias add) runs on device.

Device kernel (per core), all matmuls at float32r (full PE rate for N>=256):
  1. DMA W.T into SBUF as [128, 8, 3072]; broadcast bias to [128, 3072].
  2. Compute BbT_k[r, o] = vera_d[k][r] * vera_b[k][o] * B.T[r, o] on DVE.
  3. delta.T tiles = A.T-chunks @ BbT chunks (PE, PSUM accum over r), added
     in-place into the K/V columns of the SBUF W.T (DVE).
  4. Stream token tiles: psum[t128, o512] += xT[k, t128].T @ WT[k, o512] over
     k=0..7 (PE), bias-add psum -> SBUF (DVE), DMA out rows of y.
"""

import numpy as np

import concourse.bass as bass
import concourse.mybir as mybir
import concourse.tile as tile
from concourse import bass_utils

# ---------------------------------------------------------------------------
# Workaround: the walrus build in this container caps sync-wait commands per
# instruction, but TileContext's kernel-tail drain carries a wait for every
# logical processor (27), so codegen fails with "Too many sync wait commands"
# for ANY Tile kernel.  Split the tail-drain waits across several drain
# instructions (<=4 waits each, same sync engine => program order preserves
# the barrier semantics), then run the original epilogue without re-adding
# the full clock to a single instruction.
# ---------------------------------------------------------------------------
from bass_rust import ScopedClock as _ScopedClock, VectorClock as _VectorClock


def _split_drain_and_barrier(self, tick_clock, wait_clock):
    gc = tick_clock.global_clock
    n = len(gc)
    CH = 4
    for s in range(0, n, CH):
        vec = [0] * n
        nz = False
        for i in range(s, min(s + CH, n)):
            vec[i] = gc[i]
            nz = nz or gc[i] > 0
        if not nz:
            continue
        di = self.nc.sync.drain()
        wait_clock.add_sem_waits(di.ins, _ScopedClock({None: _VectorClock(vec)}))

    self.nc.all_engine_barrier()
    assert self.sems is not None
    popped = self.nc._tile_sem_poison_stack.pop()
    assert popped is self._sem_poison
    self.nc.clear_and_free_semaphores(list(self.sems.allocated().values()))
    self.nc.all_engine_barrier()


tile.TileContext._drain_and_barrier = _split_drain_and_barrier

N_CORES = 8
B, S = 4, 4096
I = 1024          # in features
O = 1024          # per-projection out features
O3 = 3 * O        # 3072 total out features
R = 256           # vera rank
T_TOTAL = B * S   # 16384 tokens
T = T_TOTAL // N_CORES  # 2048 tokens per core
P = 128
KO = I // P       # 8 contraction chunks
RO = R // P       # 2 rank chunks
NT = 512          # output-feature tile (one PSUM bank of fp32)
OT = O3 // NT     # 6 output tiles
TS = 512          # token DMA chunk
F32 = mybir.dt.float32
F32R = mybir.dt.float32r


def _build_kernel():
    nc = bass.Bass("TRN2", debug=False, target_bir_lowering=False)

    xT_d = nc.dram_tensor("xT", [I, T], F32, kind="ExternalInput")
    wT_d = nc.dram_tensor("wT", [I, O3], F32, kind="ExternalInput")
    bias_d = nc.dram_tensor("bias", [O3], F32, kind="ExternalInput")
    a_d = nc.dram_tensor("vera_A", [R, I], F32, kind="ExternalInput")
    bT_d = nc.dram_tensor("vera_BT", [R, O], F32, kind="ExternalInput")
    d_d = nc.dram_tensor("vera_d", [2, R], F32, kind="ExternalInput")
    b_d = nc.dram_tensor("vera_b", [2, O], F32, kind="ExternalInput")
    y_d = nc.dram_tensor("y", [T, O3], F32, kind="ExternalOutput")

    with tile.TileContext(nc) as tc:
        _kernel_body(tc, xT_d, wT_d, bias_d, a_d, bT_d, d_d, b_d, y_d)
    return nc


def _kernel_body(tc, xT_d, wT_d, bias_d, a_d, bT_d, d_d, b_d, y_d):
    nc = tc.nc
    MUL = mybir.AluOpType.mult
    ADD = mybir.AluOpType.add

    with (
        tc.tile_pool(name="persist", bufs=1) as persist,
        tc.tile_pool(name="psum", bufs=8, space="PSUM") as psum_pool,
    ):
        # W.T resident in SBUF for the whole kernel: [128(i), 8(i-chunk), 3072(o)]
        wT_sb = persist.tile([P, KO, O3], F32)
        nc.sync.dma_start(wT_sb[:], wT_d.ap().rearrange("(ko p) o -> p ko o", p=P))
        # bias broadcast to all partitions
        bias_sb = persist.tile([P, O3], F32)
        nc.sync.dma_start(bias_sb[:], bias_d.ap().partition_broadcast(P))

        # ---- VeRA delta, added in place into the K/V columns of wT_sb ----
        with tc.tile_pool(name="setup", bufs=1) as setup:
            a_sb = setup.tile([P, RO, I], F32)
            nc.sync.dma_start(a_sb[:], a_d.ap().rearrange("(ro p) i -> p ro i", p=P))
            bT_sb = setup.tile([P, RO, O], F32)
            nc.sync.dma_start(bT_sb[:], bT_d.ap().rearrange("(ro p) o -> p ro o", p=P))
            d_sb = setup.tile([P, 2, RO], F32)
            nc.sync.dma_start(d_sb[:], d_d.ap().rearrange("k (ro p) -> p k ro", p=P))
            b_bc = setup.tile([P, 2, O], F32)
            nc.sync.dma_start(b_bc[:], b_d.ap().partition_broadcast(P))

            for k in range(2):
                # BbT_k[r, o] = d[k, r] * b[k, o] * B.T[r, o]
                bbT = setup.tile([P, RO, O], F32, tag="bbT", bufs=2)
                nc.vector.tensor_tensor(
                    bbT[:], bT_sb[:],
                    d_sb[:, k, :, None].to_broadcast([P, RO, O]), MUL)
                nc.vector.tensor_tensor(
                    bbT[:], bbT[:],
                    b_bc[:, k, None, :].to_broadcast([P, RO, O]), MUL)
                # delta.T[i, o] = sum_r A[r, i] * BbT_k[r, o]
                for ic in range(KO):
                    for ot in range(O // NT):
                        pd = psum_pool.tile([P, NT], F32, tag="ps")
                        for rc in range(RO):
                            nc.tensor.matmul(
                                pd[:],
                                a_sb[:, rc, ic * P:(ic + 1) * P].bitcast(F32R),
                                bbT[:, rc, ot * NT:(ot + 1) * NT].bitcast(F32R),
                                start=(rc == 0), stop=(rc == RO - 1))
                        off = O + k * O + ot * NT
                        nc.vector.tensor_tensor(
                            wT_sb[:, ic, off:off + NT],
                            wT_sb[:, ic, off:off + NT], pd[:], ADD)

        # ---- main matmul: y[t, o] = x[t, :] @ W_eff.T + bias ----
        xT_r = xT_d.ap().rearrange("(ko p) t -> p ko t", p=P)
        with (
            tc.tile_pool(name="xpool", bufs=3) as xpool,
            tc.tile_pool(name="ypool", bufs=3) as ypool,
        ):
            for tchunk in range(T // TS):
                xt = xpool.tile([P, KO, TS], F32, tag="xt")
                nc.sync.dma_start(xt[:], xT_r[:, :, tchunk * TS:(tchunk + 1) * TS])
                for tj in range(TS // P):
                    ys = ypool.tile([P, O3], F32, tag="ys")
                    pys = [psum_pool.tile([P, NT], F32, tag="ps", name=f"py{ot}")
                           for ot in range(OT)]
                    for k in range(KO):
                        lhsT = xt[:, k, tj * P:(tj + 1) * P].bitcast(F32R)
                        for ot in range(OT):
                            nc.tensor.matmul(
                                pys[ot][:], lhsT,
                                wT_sb[:, k, ot * NT:(ot + 1) * NT].bitcast(F32R),
                                start=(k == 0), stop=(k == KO - 1))
                    for ot in range(OT):
                        nc.vector.tensor_tensor(
                            ys[:, ot * NT:(ot + 1) * NT], pys[ot][:],
                            bias_sb[:, ot * NT:(ot + 1) * NT], ADD)
                    t0 = tchunk * TS + tj * P
                    nc.sync.dma_start(y_d.ap()[t0:t0 + P, :], ys[:])


_cached_nc = None


def _get_nc():
    global _cached_nc
    if _cached_nc is None:
        _cached_nc = _build_kernel()
    return _cached_nc


def _make_in_maps(x, base_weight, base_bias, vera_A, vera_B, vera_d, vera_b):
    x2 = np.asarray(x, dtype=np.float32).reshape(T_TOTAL, I)
    wT = np.ascontiguousarray(np.asarray(base_weight, dtype=np.float32).T)
    bT = np.ascontiguousarray(np.asarray(vera_B, dtype=np.float32).T)
    bias = np.ascontiguousarray(np.asarray(base_bias, dtype=np.float32))
    a = np.ascontiguousarray(np.asarray(vera_A, dtype=np.float32))
    d = np.ascontiguousarray(np.asarray(vera_d, dtype=np.float32))
    b = np.ascontiguousarray(np.asarray(vera_b, dtype=np.float32))
    in_maps = []
    for c in range(N_CORES):
        xT_c = np.ascontiguousarray(x2[c * T:(c + 1) * T].T)
        in_maps.append({
            "xT": xT_c, "wT": wT, "bias": bias, "vera_A": a,
            "vera_BT": bT, "vera_d": d, "vera_b": b,
        })
    return in_maps


def _run_coresim(nc, in_maps):
    """Fallback: interpret the BIR per core (bit-accurate, no hardware)."""
    from concourse.bass_interp import CoreSim

    shards = []
    for in_map in in_maps:
        sim = CoreSim(nc, trace=False)
        for name, val in in_map.items():
            sim.tensor(name)[:] = val
        sim.simulate(check_with_hw=False)
        shards.append(np.array(sim.tensor("y")))
    return shards


def kernel(x, base_weight, base_bias, vera_A, vera_B, vera_d, vera_b):
    nc = _get_nc()
    in_maps = _make_in_maps(x, base_weight, base_bias, vera_A, vera_B,
                            vera_d, vera_b)
    try:
        res = bass_utils.run_bass_kernel_spmd(nc, in_maps,
                                              core_ids=list(range(N_CORES)))
        shards = [res.results[c]["y"] for c in range(N_CORES)]
    except Exception:
        # The axon PJRT execute path can be unavailable in some containers;
        # fall back to interpreting the same BIR so results stay correct.
        shards = _run_coresim(nc, in_maps)
    y = np.concatenate(shards, axis=0)
    return y.reshape(B, S, O3).astype(np.float32)



# revision 7
# speedup vs baseline: 1.4851x; 1.4851x over previous
"""Trainium2 Bass kernel for nn_LoRA_QKVlinear (VeRA-style LoRA on K/V of a QKV linear).

Reference computation (fp32):
    delta_k = diag(vera_b[k]) @ vera_B @ diag(vera_d[k]) @ vera_A   for k in {K, V}
    W_eff   = base_weight + concat([0, delta_K, delta_V], axis=0)   # (3072, 1024)
    y       = x @ W_eff.T + base_bias                               # (4, 4096, 3072)

Sharding: data-parallel over tokens (B*S = 16384 -> 2048 per core).  Each of the
8 cores gets the full (replicated) weights + vera tensors and computes the full
3072 output features for its token slice.  No collectives; host concatenates.

Host does layout-only prep (slice/transpose/bf16 cast): x -> xT per-core shard
[1024, 2048] bf16, base_weight -> W.T [1024, 3072] bf16, vera_B -> B.T bf16.
All arithmetic (vera scaling, delta matmuls, the big matmul, bias add) runs on
device; accumulation is fp32 in PSUM.  bf16 end-to-end keeps scale-relative
error ~3.5e-3 (gate 2e-2) while halving HBM traffic vs fp32.

Device schedule (per core), designed against the TRN2 timeline cost model:
  - W.T streamed as six [128, 8, 512] column tiles; x as four [128, 8, 512]
    token chunks (whole x shard stays resident: 32 KiB/partition).
  - PE starts the o-tile-0 (Q) matmuls as soon as x chunk 0 + W tile 0 land
    (~6 us) instead of waiting for all of W (~55 us in the old version).
  - Main loop is o-tile-major: for each 512-wide output tile, 16 token tiles
    x 8 k-chunk matmuls accumulate in one PSUM bank; DVE adds bias and
    converts to bf16; DMA writes the y block.
  - The VeRA delta (64 matmuls + 32 PSUM->SBUF adds into the K/V W tiles) is
    interleaved into the o-tile-1 phase two groups per token tile, so the PE
    never stalls on the DVE psum drain and the K/V tiles are patched well
    before the o-tile-2..5 phases read them.
"""

import numpy as np
import ml_dtypes

import concourse.bass as bass
import concourse.mybir as mybir
import concourse.tile as tile
from concourse import bass_utils

# ---------------------------------------------------------------------------
# Workaround: the walrus build in this container caps sync-wait commands per
# instruction, but TileContext's kernel-tail drain carries a wait for every
# logical processor (27), so codegen fails with "Too many sync wait commands"
# for ANY Tile kernel.  Split the tail-drain waits across several drain
# instructions (<=4 waits each, same sync engine => program order preserves
# the barrier semantics), then run the original epilogue without re-adding
# the full clock to a single instruction.
# ---------------------------------------------------------------------------
from bass_rust import ScopedClock as _ScopedClock, VectorClock as _VectorClock


def _split_drain_and_barrier(self, tick_clock, wait_clock):
    gc = tick_clock.global_clock
    n = len(gc)
    CH = 4
    for s in range(0, n, CH):
        vec = [0] * n
        nz = False
        for i in range(s, min(s + CH, n)):
            vec[i] = gc[i]
            nz = nz or gc[i] > 0
        if not nz:
            continue
        di = self.nc.sync.drain()
        wait_clock.add_sem_waits(di.ins, _ScopedClock({None: _VectorClock(vec)}))

    self.nc.all_engine_barrier()
    assert self.sems is not None
    popped = self.nc._tile_sem_poison_stack.pop()
    assert popped is self._sem_poison
    self.nc.clear_and_free_semaphores(list(self.sems.allocated().values()))
    self.nc.all_engine_barrier()


tile.TileContext._drain_and_barrier = _split_drain_and_barrier

N_CORES = 8
B, S = 4, 4096
I = 1024          # in features
O = 1024          # per-projection out features
O3 = 3 * O        # 3072 total out features
R = 256           # vera rank
T_TOTAL = B * S   # 16384 tokens
T = T_TOTAL // N_CORES  # 2048 tokens per core
P = 128
KO = I // P       # 8 contraction chunks
RO = R // P       # 2 rank chunks
NT = 512          # output-feature tile (one PSUM bank of fp32)
OT = O3 // NT     # 6 output tiles
TS = 512          # token DMA chunk
XC = T // TS      # 4 x chunks
TT = T // P       # 16 token tiles
F32 = mybir.dt.float32
BF16 = mybir.dt.bfloat16
BF = ml_dtypes.bfloat16


def _build_kernel():
    nc = bass.Bass("TRN2", debug=False, target_bir_lowering=False)

    xT_d = nc.dram_tensor("xT", [I, T], BF16, kind="ExternalInput")
    wT_d = nc.dram_tensor("wT", [I, O3], BF16, kind="ExternalInput")
    bias_d = nc.dram_tensor("bias", [O3], BF16, kind="ExternalInput")
    a_d = nc.dram_tensor("vera_A", [R, I], BF16, kind="ExternalInput")
    bT_d = nc.dram_tensor("vera_BT", [R, O], BF16, kind="ExternalInput")
    d_d = nc.dram_tensor("vera_d", [2, R], F32, kind="ExternalInput")
    b_d = nc.dram_tensor("vera_b", [2, O], BF16, kind="ExternalInput")
    y_d = nc.dram_tensor("y", [T, O3], BF16, kind="ExternalOutput")

    with tile.TileContext(nc) as tc:
        _kernel_body(tc, xT_d, wT_d, bias_d, a_d, bT_d, d_d, b_d, y_d)
    return nc


def _kernel_body(tc, xT_d, wT_d, bias_d, a_d, bT_d, d_d, b_d, y_d):
    nc = tc.nc
    MUL = mybir.AluOpType.mult
    ADD = mybir.AluOpType.add

    xT_r = xT_d.ap().rearrange("(ko p) t -> p ko t", p=P)
    wT_r = wT_d.ap().rearrange("(ko p) o -> p ko o", p=P)

    with (
        tc.tile_pool(name="persist", bufs=1) as persist,
        tc.tile_pool(name="psum", bufs=8, space="PSUM") as psum_pool,
        tc.tile_pool(name="ypool", bufs=20) as ypool,
    ):
        # x chunks sized [256, 256, 512, 512, 512] tokens: small leading
        # chunks let the PE start ~3us earlier.  tt -> (chunk, tile-in-chunk).
        XCHUNKS = [(0, 256), (256, 256), (512, 512), (1024, 512), (1536, 512)]
        tt_map = []
        for ci, (cst, clen) in enumerate(XCHUNKS):
            for tj in range(clen // P):
                tt_map.append((ci, tj))
        x_sb = [persist.tile([P, KO, clen], BF16, name=f"x{c}")
                for c, (_, clen) in enumerate(XCHUNKS)]
        # W tile 0 is split in column halves (DMA lands sooner); tiles 1..5
        # are whole.  w_half[j] = list of (tile, width) column pieces.
        w0a = persist.tile([P, KO, NT // 2], BF16)
        w0b = persist.tile([P, KO, NT // 2], BF16)
        w_sb = [None] + [persist.tile([P, KO, NT], BF16, name=f"w{j}")
                         for j in range(1, OT)]
        w_pieces = {0: [(w0a, 0, NT // 2), (w0b, NT // 2, NT // 2)]}
        for j in range(1, OT):
            w_pieces[j] = [(w_sb[j], 0, NT)]
        bias_sb = persist.tile([P, O3], BF16)
        a_sb = persist.tile([P, RO, I], BF16)
        bT_sb = persist.tile([P, RO, O], BF16)
        d_sb = persist.tile([P, 2, RO], F32)
        b_bc = persist.tile([P, 2, O], BF16)
        bbT = [persist.tile([P, RO, O], BF16, name=f"bbT{k}") for k in range(2)]

        # ---- input DMAs, ordered for earliest PE start on the shared DMA
        # resource: x-chunk0 + W-tile0 halves first (PE can begin ~5us),
        # vera tensors behind them, K/V W tiles last (needed latest).
        def dma_x(c):
            cst, clen = XCHUNKS[c]
            nc.sync.dma_start(x_sb[c][:], xT_r[:, :, cst:cst + clen])

        dma_x(0)
        nc.sync.dma_start(w0a[:], wT_r[:, :, 0:NT // 2])
        nc.sync.dma_start(w0b[:], wT_r[:, :, NT // 2:NT])
        dma_x(1)
        dma_x(2)
        nc.sync.dma_start(bias_sb[:], bias_d.ap().partition_broadcast(P))
        nc.sync.dma_start(d_sb[:], d_d.ap().rearrange("k (ro p) -> p k ro", p=P))
        nc.sync.dma_start(a_sb[:], a_d.ap().rearrange("(ro p) i -> p ro i", p=P))
        nc.sync.dma_start(bT_sb[:], bT_d.ap().rearrange("(ro p) o -> p ro o", p=P))
        nc.sync.dma_start(b_bc[:], b_d.ap().partition_broadcast(P))
        dma_x(3)
        dma_x(4)
        nc.sync.dma_start(w_sb[1][:], wT_r[:, :, NT:2 * NT])
        for j in range(2, OT):
            nc.sync.dma_start(w_sb[j][:], wT_r[:, :, j * NT:(j + 1) * NT])

        def main_tile(ot, tt):
            """One (o-tile, token-tile): 8 accumulating matmuls per column
            piece, bias add (fp32 psum -> bf16), DMA the y block out."""
            ps = psum_pool.tile([P, NT], F32, tag="ps", name=f"ps_{ot}_{tt}")
            ci, tj = tt_map[tt]
            lhs_x = x_sb[ci]
            for (wt, coff, cw) in w_pieces[ot]:
                for k in range(KO):
                    nc.tensor.matmul(
                        ps[:, coff:coff + cw],
                        lhs_x[:, k, tj * P:(tj + 1) * P],
                        wt[:, k, :],
                        start=(k == 0), stop=(k == KO - 1))
            ys = ypool.tile([P, NT], BF16, tag="ys", name=f"ys_{ot}_{tt}")
            nc.vector.tensor_tensor(
                ys[:], ps[:], bias_sb[:, ot * NT:(ot + 1) * NT], ADD)
            nc.sync.dma_start(
                y_d.ap()[tt * P:(tt + 1) * P, ot * NT:(ot + 1) * NT], ys[:])

        def delta_group(k, ic, oth):
            """delta.T[i-block ic, o-half oth] for projection k, accumulated
            over both rank chunks, then added into the K/V W tile."""
            pd = psum_pool.tile([P, NT], F32, tag="ps", name=f"pd_{k}_{ic}_{oth}")
            for rc in range(RO):
                nc.tensor.matmul(
                    pd[:],
                    a_sb[:, rc, ic * P:(ic + 1) * P],
                    bbT[k][:, rc, oth * NT:(oth + 1) * NT],
                    start=(rc == 0), stop=(rc == RO - 1))
            wj = w_sb[2 + 2 * k + oth]
            nc.vector.tensor_tensor(wj[:, ic, :], wj[:, ic, :], pd[:], ADD)

        def emit_bbT():
            # bbT_k[r, o] = d[k, r] * b[k, o] * B.T[r, o]  (DVE, bf16 out)
            for k in range(2):
                nc.vector.tensor_tensor(
                    bbT[k][:], bT_sb[:],
                    d_sb[:, k, :, None].to_broadcast([P, RO, O]), MUL)
                nc.vector.tensor_tensor(
                    bbT[k][:], bbT[k][:],
                    b_bc[:, k, None, :].to_broadcast([P, RO, O]), MUL)

        # ---- phase ot0 (Q first half): PE warms up while the rest of the
        # inputs stream in.  bbT emitted after tt3 so the DVE queue first
        # drains the early psum evictions.
        for tt in range(TT):
            main_tile(0, tt)
            if tt == 3:
                emit_bbT()

        # ---- phase ot1 (Q second half) with the 32 delta groups woven in,
        # two per token tile: PE alternates 2 delta MM pairs + 8 main MMs;
        # DVE drains one delta psum per ~2.5us of PE work, so no stalls.
        groups = [(k, ic, oth) for k in range(2) for ic in range(KO)
                  for oth in range(2)]
        for tt in range(TT):
            for g in groups[2 * tt:2 * tt + 2]:
                delta_group(*g)
            main_tile(1, tt)

        # ---- phases ot2..ot5 (K then V, now patched with the vera delta)
        for ot in range(2, OT):
            for tt in range(TT):
                main_tile(ot, tt)


_cached_nc = None


def _get_nc():
    global _cached_nc
    if _cached_nc is None:
        _cached_nc = _build_kernel()
    return _cached_nc


def _make_in_maps(x, base_weight, base_bias, vera_A, vera_B, vera_d, vera_b):
    x2 = np.asarray(x, dtype=np.float32).reshape(T_TOTAL, I)
    wT = np.ascontiguousarray(np.asarray(base_weight, dtype=np.float32).T.astype(BF))
    bT = np.ascontiguousarray(np.asarray(vera_B, dtype=np.float32).T.astype(BF))
    bias = np.ascontiguousarray(np.asarray(base_bias, dtype=np.float32).astype(BF))
    a = np.ascontiguousarray(np.asarray(vera_A, dtype=np.float32).astype(BF))
    d = np.ascontiguousarray(np.asarray(vera_d, dtype=np.float32))
    b = np.ascontiguousarray(np.asarray(vera_b, dtype=np.float32).astype(BF))
    in_maps = []
    for c in range(N_CORES):
        xT_c = np.ascontiguousarray(x2[c * T:(c + 1) * T].T.astype(BF))
        in_maps.append({
            "xT": xT_c, "wT": wT, "bias": bias, "vera_A": a,
            "vera_BT": bT, "vera_d": d, "vera_b": b,
        })
    return in_maps


def _run_coresim(nc, in_maps):
    """Fallback: interpret the BIR per core (bit-accurate, no hardware)."""
    from concourse.bass_interp import CoreSim

    shards = []
    for in_map in in_maps:
        sim = CoreSim(nc, trace=False)
        for name, val in in_map.items():
            sim.tensor(name)[:] = val
        sim.simulate(check_with_hw=False)
        shards.append(np.array(sim.tensor("y")))
    return shards


def kernel(x, base_weight, base_bias, vera_A, vera_B, vera_d, vera_b):
    nc = _get_nc()
    in_maps = _make_in_maps(x, base_weight, base_bias, vera_A, vera_B,
                            vera_d, vera_b)
    try:
        res = bass_utils.run_bass_kernel_spmd(nc, in_maps,
                                              core_ids=list(range(N_CORES)))
        shards = [np.asarray(res.results[c]["y"]) for c in range(N_CORES)]
    except Exception:
        # The axon PJRT execute path can be unavailable in some containers;
        # fall back to interpreting the same BIR so results stay correct.
        shards = _run_coresim(nc, in_maps)
    y = np.concatenate(shards, axis=0).astype(np.float32)
    return y.reshape(B, S, O3)


# revision 22
# speedup vs baseline: 1.7918x; 1.2065x over previous
"""Trainium2 Bass kernel for nn_LoRA_QKVlinear (VeRA-style LoRA on K/V of a QKV linear).

Reference computation (fp32):
    delta_k = diag(vera_b[k]) @ vera_B @ diag(vera_d[k]) @ vera_A   for k in {K, V}
    W_eff   = base_weight + concat([0, delta_K, delta_V], axis=0)   # (3072, 1024)
    y       = x @ W_eff.T + base_bias                               # (4, 4096, 3072)

Sharding: data-parallel over tokens (B*S = 16384 -> 2048 per core).  Each of the
8 cores gets the full (replicated) weights + vera tensors and computes the full
3072 output features for its token slice.  No collectives; host concatenates.

Precision: mixed bf16 / fp8.  Six of the eight 128-deep contraction chunks run
in bf16; the last two run as a single fp8(e4m3) DoubleRow matmul (two packed
k-tiles, half the PE cycles per row).  The fp8 operands are rescaled so both
sit in e4m3's normal range (x/8 and 8*W -- the product is exact in scale), and
the VeRA delta for those chunks folds in for free because the matching columns
of vera_A are pre-scaled by 8 on the host.  All accumulation is fp32 in PSUM.
Measured scale-relative error 1.55e-2 vs the 2e-2 gate (pure bf16: 3.5e-3,
pure fp8: 2.8e-2 -- fails).

Device schedule (per core), designed against the TRN2 timeline cost model
(PE runs ~147 us of back-to-back full-rate matmuls -- the mixed-precision
floor -- with zero mid-stream stalls):
  - W.T streamed as column tiles (tile 0 in two 256-wide halves so it lands
    sooner); bf16 x as five token chunks [256, 256, 512, 512, 512]; the fp8
    x chunk and the whole x shard stay resident in SBUF.
  - Eight dummy matmuls on a zeroed tile run while the first DMAs are in
    flight, so the PE p-state ramp (half clock for the first ~5.5 us of
    activity) is burned on throwaway work and every real matmul runs at the
    full 2.4 GHz rate.
  - Main loop is o-tile-major: for each 512-wide output tile, 16 token tiles
    x (1 DoubleRow fp8 + 6 bf16) matmuls accumulate in one PSUM bank; DVE
    adds bias and converts to bf16; DMA writes the y block.  Input DMAs are
    hand-ordered on the (serialized) DMA resource; 20 ys staging buffers
    decouple PSUM eviction from the y write-back queue.
  - The VeRA delta (64 matmuls + 32 PSUM->SBUF adds) is interleaved into the
    o-tile-1 phase two groups per token tile.  Chunks 0..5 add into the bf16
    K/V W tiles; chunks 6..7 combine the (8x-scaled) delta with the 8x-scaled
    bf16 base rows and write the fp8 K/V W tiles in a single rounding.
  - Final tile computed as two independent PSUM halves, halving the
    kernel-end evict->DMA->semaphore chain.
"""

import numpy as np
import ml_dtypes

import concourse.bass as bass
import concourse.mybir as mybir
import concourse.tile as tile
from concourse import bass_utils

# ---------------------------------------------------------------------------
# Workaround: the walrus build in this container caps sync-wait commands per
# instruction, but TileContext's kernel-tail drain carries a wait for every
# logical processor (27), so codegen fails with "Too many sync wait commands"
# for ANY Tile kernel.  Split the tail-drain waits across several drain
# instructions (<=4 waits each, same sync engine => program order preserves
# the barrier semantics), then run the original epilogue without re-adding
# the full clock to a single instruction.
# ---------------------------------------------------------------------------
from bass_rust import ScopedClock as _ScopedClock, VectorClock as _VectorClock


def _split_drain_and_barrier(self, tick_clock, wait_clock):
    gc = tick_clock.global_clock
    n = len(gc)
    CH = 4
    for s in range(0, n, CH):
        vec = [0] * n
        nz = False
        for i in range(s, min(s + CH, n)):
            vec[i] = gc[i]
            nz = nz or gc[i] > 0
        if not nz:
            continue
        di = self.nc.sync.drain()
        wait_clock.add_sem_waits(di.ins, _ScopedClock({None: _VectorClock(vec)}))

    self.nc.all_engine_barrier()
    assert self.sems is not None
    popped = self.nc._tile_sem_poison_stack.pop()
    assert popped is self._sem_poison
    self.nc.clear_and_free_semaphores(list(self.sems.allocated().values()))
    self.nc.all_engine_barrier()


tile.TileContext._drain_and_barrier = _split_drain_and_barrier

N_CORES = 8
B, S = 4, 4096
I = 1024          # in features
O = 1024          # per-projection out features
O3 = 3 * O        # 3072 total out features
R = 256           # vera rank
T_TOTAL = B * S   # 16384 tokens
T = T_TOTAL // N_CORES  # 2048 tokens per core
P = 128
KO = I // P       # 8 contraction chunks
KB = 6            # bf16 contraction chunks (0..5)
KF = 2            # fp8 contraction chunks (6..7), one DoubleRow matmul
IB = KB * P       # 768 bf16 in-features
RO = R // P       # 2 rank chunks
NT = 512          # output-feature tile (one PSUM bank of fp32)
OT = O3 // NT     # 6 output tiles
TT = T // P       # 16 token tiles
SWX = 5.66      # fp8 rescale: x/SWX and W*SWX keep e4m3 in its normal range
F32 = mybir.dt.float32
BF16 = mybir.dt.bfloat16
FP8 = mybir.dt.float8e4
BF = ml_dtypes.bfloat16
F8 = ml_dtypes.float8_e4m3
DR = mybir.MatmulPerfMode.DoubleRow


def _build_kernel():
    nc = bass.Bass("TRN2", debug=False, target_bir_lowering=False)

    xT_d = nc.dram_tensor("xT", [IB, T], BF16, kind="ExternalInput")
    x8_d = nc.dram_tensor("x8T", [KF * P, T], FP8, kind="ExternalInput")
    wT_d = nc.dram_tensor("wT", [IB, O3], BF16, kind="ExternalInput")
    w8q_d = nc.dram_tensor("w8q", [KF * P, 2 * NT], FP8, kind="ExternalInput")
    wb67_d = nc.dram_tensor("wb67", [KF * P, 2 * O], BF16, kind="ExternalInput")
    bias_d = nc.dram_tensor("bias", [O3], BF16, kind="ExternalInput")
    a_d = nc.dram_tensor("vera_A", [R, I], BF16, kind="ExternalInput")
    bT_d = nc.dram_tensor("vera_BT", [R, O], BF16, kind="ExternalInput")
    d_d = nc.dram_tensor("vera_d", [2, R], F32, kind="ExternalInput")
    b_d = nc.dram_tensor("vera_b", [2, O], BF16, kind="ExternalInput")
    y_d = nc.dram_tensor("y", [T, O3], BF16, kind="ExternalOutput")

    with tile.TileContext(nc) as tc:
        _kernel_body(tc, xT_d, x8_d, wT_d, w8q_d, wb67_d, bias_d, a_d, bT_d,
                     d_d, b_d, y_d)
    return nc


def _kernel_body(tc, xT_d, x8_d, wT_d, w8q_d, wb67_d, bias_d, a_d, bT_d,
                 d_d, b_d, y_d):
    nc = tc.nc
    MUL = mybir.AluOpType.mult
    ADD = mybir.AluOpType.add

    xT_r = xT_d.ap().rearrange("(ko p) t -> p ko t", p=P)
    x8_r = x8_d.ap().rearrange("(ko p) t -> p ko t", p=P)
    wT_r = wT_d.ap().rearrange("(ko p) o -> p ko o", p=P)
    w8q_r = w8q_d.ap().rearrange("(ko p) o -> p ko o", p=P)
    wb67_r = wb67_d.ap().rearrange("(ko p) o -> p ko o", p=P)

    with (
        tc.tile_pool(name="persist", bufs=1) as persist,
        tc.tile_pool(name="psum", bufs=8, space="PSUM") as psum_pool,
        tc.tile_pool(name="ypool", bufs=20) as ypool,
    ):
        # bf16 x chunks sized [256, 256, 512, 512, 512] tokens: small leading
        # chunks let the PE start ~3us earlier.  tt -> (chunk, tile-in-chunk).
        XCHUNKS = [(0, 256), (256, 256), (512, 512), (1024, 512), (1536, 512)]
        tt_map = []
        for ci, (cst, clen) in enumerate(XCHUNKS):
            for tj in range(clen // P):
                tt_map.append((ci, tj))
        x_sb = [persist.tile([P, KB, clen], BF16, name=f"x{c}")
                for c, (_, clen) in enumerate(XCHUNKS)]
        x8_sb = persist.tile([P, KF, T], FP8)   # whole fp8 x shard resident
        # bf16 W tile 0 is split in column halves (DMA lands sooner); tiles
        # 1..5 whole.  w_pieces[j] = list of (tile, col offset, width).
        w0a = persist.tile([P, KB, NT // 2], BF16)
        w0b = persist.tile([P, KB, NT // 2], BF16)
        w_sb = [None] + [persist.tile([P, KB, NT], BF16, name=f"w{j}")
                         for j in range(1, OT)]
        w_pieces = {0: [(w0a, 0, NT // 2), (w0b, NT // 2, NT // 2)]}
        for j in range(1, OT):
            w_pieces[j] = [(w_sb[j], 0, NT)]
        # fp8 W tiles (k-chunks 6..7): Q tiles DMA'd from the host, K/V tiles
        # written on device by the delta adds.
        w8_sb = [persist.tile([P, KF, NT], FP8, name=f"w8_{j}")
                 for j in range(OT)]
        wb67_sb = persist.tile([P, KF, 2 * O], BF16)  # 8x-scaled K/V base rows
        bias_sb = persist.tile([P, O3], BF16)
        a_sb = persist.tile([P, RO, I], BF16)
        bT_sb = persist.tile([P, RO, O], BF16)
        d_sb = persist.tile([P, 2, RO], F32)
        b_bc = persist.tile([P, 2, O], BF16)
        bbT = [persist.tile([P, RO, O], BF16, name=f"bbT{k}") for k in range(2)]
        warm_sb = persist.tile([P, NT], BF16)

        # ---- PE pre-warm: zero a dummy tile, then issue matmuls on it so
        # the PE p-state ramp (full clock only after ~5.5us of sustained
        # busy in the cost model) burns off while the first input DMAs are
        # in flight; the real matmuls then start at full rate.
        nc.vector.memset(warm_sb[:], 0.0)
        warm_ps = psum_pool.tile([P, NT], F32, tag="ps", name="warm_ps")
        for _ in range(8):
            nc.tensor.matmul(warm_ps[:], warm_sb[:, 0:P], warm_sb[:],
                             start=True, stop=True)

        # ---- input DMAs, ordered for earliest PE start on the shared DMA
        # resource: x-chunk0 + W-tile0 pieces first, the rest of x/W next,
        # vera tensors behind them, K/V weight rows last (needed latest).
        def dma_x(c):
            cst, clen = XCHUNKS[c]
            nc.sync.dma_start(x_sb[c][:], xT_r[:, :, cst:cst + clen])

        dma_x(0)
        nc.sync.dma_start(w0a[:], wT_r[:, :, 0:NT // 2])
        nc.sync.dma_start(x8_sb[:], x8_r[:])
        nc.sync.dma_start(w8_sb[0][:], w8q_r[:, :, 0:NT])
        nc.sync.dma_start(w0b[:], wT_r[:, :, NT // 2:NT])
        dma_x(1)
        dma_x(2)
        nc.sync.dma_start(bias_sb[:], bias_d.ap().partition_broadcast(P))
        dma_x(3)
        dma_x(4)
        nc.sync.dma_start(d_sb[:], d_d.ap().rearrange("k (ro p) -> p k ro", p=P))
        nc.sync.dma_start(a_sb[:], a_d.ap().rearrange("(ro p) i -> p ro i", p=P))
        nc.sync.dma_start(bT_sb[:], bT_d.ap().rearrange("(ro p) o -> p ro o", p=P))
        nc.sync.dma_start(b_bc[:], b_d.ap().partition_broadcast(P))
        nc.sync.dma_start(w_sb[1][:], wT_r[:, :, NT:2 * NT])
        nc.sync.dma_start(w8_sb[1][:], w8q_r[:, :, NT:2 * NT])
        nc.sync.dma_start(wb67_sb[:], wb67_r[:])
        for j in range(2, OT):
            nc.sync.dma_start(w_sb[j][:], wT_r[:, :, j * NT:(j + 1) * NT])

        def mm_group(ps, ot, tt, wt, coff, cw):
            """The accumulation group for psum region [coff, coff+cw): one
            fp8 DoubleRow matmul (k-chunks 6..7) + KB bf16 matmuls on the
            bf16 W piece `wt` (whose columns span that same region)."""
            ci, tj = tt_map[tt]
            lhs_x = x_sb[ci]
            nc.tensor.matmul(
                ps[:, coff:coff + cw],
                x8_sb[:, :, tt * P:(tt + 1) * P],
                w8_sb[ot][:, :, coff:coff + cw],
                start=True, stop=False, perf_mode=DR)
            for k in range(KB):
                nc.tensor.matmul(
                    ps[:, coff:coff + cw],
                    lhs_x[:, k, tj * P:(tj + 1) * P],
                    wt[:, k, 0:cw],
                    start=False, stop=(k == KB - 1))

        def main_tile(ot, tt):
            """One (o-tile, token-tile): fp8 DR + bf16 accumulating matmuls
            per column piece, bias add (fp32 psum -> bf16), DMA y out."""
            if ot == OT - 1 and tt == TT - 1:
                # final tile: two independent psum tiles so the first half's
                # evict + DMA drain during the second half's matmuls, halving
                # the kernel-end chain after the very last matmul.
                h = NT // 2
                for c in range(2):
                    psh = psum_pool.tile([P, h], F32, tag="ps",
                                         name=f"ps_last_{c}")
                    nc.tensor.matmul(
                        psh[:],
                        x8_sb[:, :, tt * P:(tt + 1) * P],
                        w8_sb[ot][:, :, c * h:(c + 1) * h],
                        start=True, stop=False, perf_mode=DR)
                    ci, tj = tt_map[tt]
                    (wt, _, _), = w_pieces[ot]
                    for k in range(KB):
                        nc.tensor.matmul(
                            psh[:],
                            x_sb[ci][:, k, tj * P:(tj + 1) * P],
                            wt[:, k, c * h:(c + 1) * h],
                            start=False, stop=(k == KB - 1))
                    o0 = ot * NT + c * h
                    ysh = ypool.tile([P, h], BF16, tag="ys", name=f"ys_last_{c}")
                    nc.vector.tensor_tensor(
                        ysh[:], psh[:], bias_sb[:, o0:o0 + h], ADD)
                    nc.sync.dma_start(
                        y_d.ap()[tt * P:(tt + 1) * P, o0:o0 + h], ysh[:])
                return
            ps = psum_pool.tile([P, NT], F32, tag="ps", name=f"ps_{ot}_{tt}")
            ys = ypool.tile([P, NT], BF16, tag="ys", name=f"ys_{ot}_{tt}")
            for (wt, coff, cw) in w_pieces[ot]:
                mm_group(ps, ot, tt, wt, coff, cw)
            nc.vector.tensor_tensor(
                ys[:], ps[:], bias_sb[:, ot * NT:(ot + 1) * NT], ADD)
            nc.sync.dma_start(
                y_d.ap()[tt * P:(tt + 1) * P, ot * NT:(ot + 1) * NT], ys[:])

        def delta_group(k, ic, oth):
            """delta.T[i-block ic, o-half oth] for projection k, accumulated
            over both rank chunks, then folded into the K/V weights: bf16
            chunks add in place; fp8 chunks (ic 6..7, 8x-scaled via the host-
            scaled vera_A columns) combine with the 8x-scaled bf16 base rows
            and write the fp8 tile in one rounding."""
            pd = psum_pool.tile([P, NT], F32, tag="ps", name=f"pd_{k}_{ic}_{oth}")
            for rc in range(RO):
                nc.tensor.matmul(
                    pd[:],
                    a_sb[:, rc, ic * P:(ic + 1) * P],
                    bbT[k][:, rc, oth * NT:(oth + 1) * NT],
                    start=(rc == 0), stop=(rc == RO - 1))
            j = 2 + 2 * k + oth
            if ic < KB:
                wj = w_sb[j]
                nc.vector.tensor_tensor(wj[:, ic, :], wj[:, ic, :], pd[:], ADD)
            else:
                icf = ic - KB
                boff = k * O + oth * NT
                nc.vector.tensor_tensor(
                    w8_sb[j][:, icf, :],
                    wb67_sb[:, icf, boff:boff + NT], pd[:], ADD)

        def emit_bbT():
            # bbT_k[r, o] = d[k, r] * b[k, o] * B.T[r, o]  (DVE, bf16 out)
            for k in range(2):
                nc.vector.tensor_tensor(
                    bbT[k][:], bT_sb[:],
                    d_sb[:, k, :, None].to_broadcast([P, RO, O]), MUL)
                nc.vector.tensor_tensor(
                    bbT[k][:], bbT[k][:],
                    b_bc[:, k, None, :].to_broadcast([P, RO, O]), MUL)

        # ---- phase ot0 (Q first half): PE warms up while the rest of the
        # inputs stream in.  bbT emitted after tt3 so the DVE queue first
        # drains the early psum evictions.
        for tt in range(TT):
            main_tile(0, tt)
            if tt == 3:
                emit_bbT()

        # ---- phase ot1 (Q second half) with the 32 delta groups woven in,
        # two per token tile: PE alternates 2 delta MM pairs + main MMs;
        # DVE drains one delta psum per ~2us of PE work, so no stalls.
        groups = [(k, ic, oth) for k in range(2) for ic in range(KO)
                  for oth in range(2)]
        for tt in range(TT):
            for g in groups[2 * tt:2 * tt + 2]:
                delta_group(*g)
            main_tile(1, tt)

        # ---- phases ot2..ot5 (K then V, now patched with the vera delta)
        for ot in range(2, OT):
            for tt in range(TT):
                main_tile(ot, tt)


_cached_nc = None


def _get_nc():
    global _cached_nc
    if _cached_nc is None:
        _cached_nc = _build_kernel()
    return _cached_nc


def _make_in_maps(x, base_weight, base_bias, vera_A, vera_B, vera_d, vera_b):
    x2 = np.asarray(x, dtype=np.float32).reshape(T_TOTAL, I)
    Wb = np.asarray(base_weight, dtype=np.float32)
    wT = np.ascontiguousarray(Wb[:, :IB].T.astype(BF))
    # fp8 side: Q rows as fp8(8*W); K/V rows as bf16(8*W) (the device adds the
    # 8x-scaled delta and performs the single fp8 rounding itself)
    w8q = np.ascontiguousarray((Wb[:O, IB:] * SWX).T.astype(F8))
    wb67 = np.ascontiguousarray(
        (Wb[O:, IB:].astype(BF).astype(np.float32).T * SWX).astype(BF))
    bT = np.ascontiguousarray(np.asarray(vera_B, dtype=np.float32).T.astype(BF))
    bias = np.ascontiguousarray(np.asarray(base_bias, dtype=np.float32).astype(BF))
    a = np.asarray(vera_A, dtype=np.float32).copy()
    a[:, IB:] *= SWX          # pre-scale so the fp8-chunk delta lands 8x-scaled
    a = np.ascontiguousarray(a.astype(BF))
    d = np.ascontiguousarray(np.asarray(vera_d, dtype=np.float32))
    b = np.ascontiguousarray(np.asarray(vera_b, dtype=np.float32).astype(BF))
    in_maps = []
    for c in range(N_CORES):
        xs = x2[c * T:(c + 1) * T]
        xT_c = np.ascontiguousarray(xs[:, :IB].T.astype(BF))
        x8_c = np.ascontiguousarray((xs[:, IB:] / SWX).T.astype(F8))
        in_maps.append({
            "xT": xT_c, "x8T": x8_c, "wT": wT, "w8q": w8q, "wb67": wb67,
            "bias": bias, "vera_A": a, "vera_BT": bT, "vera_d": d, "vera_b": b,
        })
    return in_maps


def _run_coresim(nc, in_maps):
    """Fallback: interpret the BIR per core (bit-accurate, no hardware)."""
    from concourse.bass_interp import CoreSim

    shards = []
    for in_map in in_maps:
        sim = CoreSim(nc, trace=False)
        for name, val in in_map.items():
            sim.tensor(name)[:] = val
        sim.simulate(check_with_hw=False)
        shards.append(np.array(sim.tensor("y")))
    return shards


def kernel(x, base_weight, base_bias, vera_A, vera_B, vera_d, vera_b):
    nc = _get_nc()
    in_maps = _make_in_maps(x, base_weight, base_bias, vera_A, vera_B,
                            vera_d, vera_b)
    try:
        res = bass_utils.run_bass_kernel_spmd(nc, in_maps,
                                              core_ids=list(range(N_CORES)))
        shards = [np.asarray(res.results[c]["y"]) for c in range(N_CORES)]
    except Exception:
        # The axon PJRT execute path can be unavailable in some containers;
        # fall back to interpreting the same BIR so results stay correct.
        shards = _run_coresim(nc, in_maps)
    y = np.concatenate(shards, axis=0).astype(np.float32)
    return y.reshape(B, S, O3)


# revision 49
# speedup vs baseline: 1.9404x; 1.0829x over previous
"""Trainium2 Bass kernel for nn_LoRA_QKVlinear (VeRA-style LoRA on K/V of a QKV linear).

Reference computation (fp32):
    delta_k = diag(vera_b[k]) @ vera_B @ diag(vera_d[k]) @ vera_A   for k in {K, V}
    W_eff   = base_weight + concat([0, delta_K, delta_V], axis=0)   # (3072, 1024)
    y       = x @ W_eff.T + base_bias                               # (4, 4096, 3072)

Sharding: data-parallel over tokens (B*S = 16384 -> 2048 per core).  Each of the
8 cores gets the full (replicated) weights + vera tensors and computes the full
3072 output features for its token slice.  No collectives; host concatenates.

Precision: mixed bf16 / fp8.  Six of the eight 128-deep contraction chunks run
in bf16; the last two run as a single fp8(e4m3) DoubleRow matmul (two packed
k-tiles, half the PE cycles per row).  The fp8 operands are rescaled so both
sit in e4m3's normal range (x/SWX and SWX*W -- the product is exact in scale),
and the VeRA delta for those chunks folds in for free because the matching
columns of vera_A are pre-scaled by SWX on the host.  All accumulation is fp32
in PSUM.  Measured scale-relative error 1.56e-2 vs the 2e-2 gate (pure bf16:
3.5e-3, pure fp8: 2.8e-2 -- fails).

Device schedule (per core), designed against the TRN2 timeline cost model
(~146 us span: ~136.5 us of back-to-back full-rate matmuls -- the
mixed-precision floor -- plus DMA/p-state head and drain tail; zero
mid-stream stalls):
  - W.T streamed as column tiles (tile 0 in two 256-wide halves so it lands
    sooner); bf16 x as five token chunks [256, 256, 512, 512, 512] and fp8
    x as two chunks [512, 1536]; the whole x shard stays resident in SBUF.
  - Eight dummy matmuls on a zeroed tile run while the first DMAs are in
    flight, so the PE p-state ramp (half clock for the first ~5.5 us of
    activity) is burned on throwaway work and every real matmul runs at the
    full 2.4 GHz rate.
  - Main loop is o-tile-major: for each 512-wide output tile, 16 token tiles
    x (1 DoubleRow fp8 + 6 bf16) matmuls accumulate in one PSUM bank; DVE
    adds bias and converts to bf16; DMA writes the y block.  Input DMAs are
    hand-ordered on the (serialized) DMA resource; 20 ys staging buffers
    decouple PSUM eviction from the y write-back queue.
  - The VeRA delta also runs in fp8 DoubleRow (one matmul per i-block/o-half
    over both rank chunks; operands pre-scaled by 32 and 256, with the 8192x
    descale fused into the weight fold via scalar_tensor_tensor on the
    otherwise-idle GPSIMD engine, keeping the DVE free for psum eviction).
    K groups weave into the o-tile-1 phase, V groups into o-tile-2, one per
    token tile, so no engine stalls another.  Chunks 0..5 add into the bf16 K/V W
    tiles; chunks 6..7 combine the delta with the SWX-scaled bf16 base rows
    and write the fp8 K/V tiles in one rounding.
  - Final tile computed as two independent PSUM halves, halving the
    kernel-end evict->DMA->semaphore chain.
"""

import numpy as np
import ml_dtypes

import concourse.bass as bass
import concourse.mybir as mybir
import concourse.tile as tile
from concourse import bass_utils

# ---------------------------------------------------------------------------
# Workaround: the walrus build in this container caps sync-wait commands per
# instruction, but TileContext's kernel-tail drain carries a wait for every
# logical processor (27), so codegen fails with "Too many sync wait commands"
# for ANY Tile kernel.  Split the tail-drain waits across several drain
# instructions (<=4 waits each, same sync engine => program order preserves
# the barrier semantics).  The epilogue keeps the post-drain engine barrier
# (all work complete, output final in DRAM) but drops the semaphore
# clear-and-free pass + second barrier: this kernel is single-shot per NEFF
# load, so sem cleanup for re-execution is dead time (~0.3us/core).
# ---------------------------------------------------------------------------
from bass_rust import ScopedClock as _ScopedClock, VectorClock as _VectorClock


def _split_drain_and_barrier(self, tick_clock, wait_clock):
    gc = tick_clock.global_clock
    n = len(gc)
    CH = 4
    for s in range(0, n, CH):
        vec = [0] * n
        nz = False
        for i in range(s, min(s + CH, n)):
            vec[i] = gc[i]
            nz = nz or gc[i] > 0
        if not nz:
            continue
        di = self.nc.sync.drain()
        wait_clock.add_sem_waits(di.ins, _ScopedClock({None: _VectorClock(vec)}))

    self.nc.all_engine_barrier()
    assert self.sems is not None
    popped = self.nc._tile_sem_poison_stack.pop()
    assert popped is self._sem_poison


tile.TileContext._drain_and_barrier = _split_drain_and_barrier

N_CORES = 8
B, S = 4, 4096
I = 1024          # in features
O = 1024          # per-projection out features
O3 = 3 * O        # 3072 total out features
R = 256           # vera rank
T_TOTAL = B * S   # 16384 tokens
T = T_TOTAL // N_CORES  # 2048 tokens per core
P = 128
KO = I // P       # 8 contraction chunks
KB = 6            # bf16 contraction chunks (0..5)
KF = 2            # fp8 contraction chunks (6..7), one DoubleRow matmul
IB = KB * P       # 768 bf16 in-features
RO = R // P       # 2 rank chunks
NT = 512          # output-feature tile (one PSUM bank of fp32)
OT = O3 // NT     # 6 output tiles
TT = T // P       # 16 token tiles
SWX = 5.66      # fp8 rescale: x/SWX and W*SWX keep e4m3 in its normal range
F32 = mybir.dt.float32
BF16 = mybir.dt.bfloat16
FP8 = mybir.dt.float8e4
BF = ml_dtypes.bfloat16
F8 = ml_dtypes.float8_e4m3
DR = mybir.MatmulPerfMode.DoubleRow


def _build_kernel():
    nc = bass.Bass("TRN2", debug=False, target_bir_lowering=False)

    xT_d = nc.dram_tensor("xT", [IB, T], BF16, kind="ExternalInput")
    x8_d = nc.dram_tensor("x8T", [KF * P, T], FP8, kind="ExternalInput")
    wT_d = nc.dram_tensor("wT", [IB, O3], BF16, kind="ExternalInput")
    w8q_d = nc.dram_tensor("w8q", [KF * P, 2 * NT], FP8, kind="ExternalInput")
    wb67_d = nc.dram_tensor("wb67", [KF * P, 2 * O], BF16, kind="ExternalInput")
    bias_d = nc.dram_tensor("bias", [O3], BF16, kind="ExternalInput")
    a_d = nc.dram_tensor("vera_A", [R, I], FP8, kind="ExternalInput")
    bT_d = nc.dram_tensor("vera_BT", [R, O], BF16, kind="ExternalInput")
    d_d = nc.dram_tensor("vera_d", [2, R], F32, kind="ExternalInput")
    b_d = nc.dram_tensor("vera_b", [2, O], BF16, kind="ExternalInput")
    y_d = nc.dram_tensor("y", [T, O3], BF16, kind="ExternalOutput")

    with tile.TileContext(nc) as tc:
        _kernel_body(tc, xT_d, x8_d, wT_d, w8q_d, wb67_d, bias_d, a_d, bT_d,
                     d_d, b_d, y_d)
    return nc


def _kernel_body(tc, xT_d, x8_d, wT_d, w8q_d, wb67_d, bias_d, a_d, bT_d,
                 d_d, b_d, y_d):
    nc = tc.nc
    MUL = mybir.AluOpType.mult
    ADD = mybir.AluOpType.add

    xT_r = xT_d.ap().rearrange("(ko p) t -> p ko t", p=P)
    x8_r = x8_d.ap().rearrange("(ko p) t -> p ko t", p=P)
    wT_r = wT_d.ap().rearrange("(ko p) o -> p ko o", p=P)
    w8q_r = w8q_d.ap().rearrange("(ko p) o -> p ko o", p=P)
    wb67_r = wb67_d.ap().rearrange("(ko p) o -> p ko o", p=P)

    with (
        tc.tile_pool(name="persist", bufs=1) as persist,
        tc.tile_pool(name="psum", bufs=8, space="PSUM") as psum_pool,
        tc.tile_pool(name="ypool", bufs=20) as ypool,
    ):
        # bf16 x chunks sized [256, 256, 512, 512, 512] tokens: small leading
        # chunks let the PE start ~3us earlier.  tt -> (chunk, tile-in-chunk).
        XCHUNKS = [(0, 256), (256, 256), (512, 512), (1024, 512), (1536, 512)]
        tt_map = []
        for ci, (cst, clen) in enumerate(XCHUNKS):
            for tj in range(clen // P):
                tt_map.append((ci, tj))
        x_sb = [persist.tile([P, KB, clen], BF16, name=f"x{c}")
                for c, (_, clen) in enumerate(XCHUNKS)]
        X8CHUNKS = [(0, 512), (512, 1536)]   # fp8 x chunks (tokens)
        x8_sb = [persist.tile([P, KF, clen], FP8, name=f"x8_{c}")
                 for c, (_, clen) in enumerate(X8CHUNKS)]

        def x8_slab(tt):
            c = 0 if tt < 4 else 1
            lo = tt * P - X8CHUNKS[c][0]
            return x8_sb[c][:, :, lo:lo + P]
        # bf16 W tile 0 is split in column halves (DMA lands sooner); tiles
        # 1..5 whole.  w_pieces[j] = list of (tile, col offset, width).
        w0a = persist.tile([P, KB, NT // 2], BF16)
        w0b = persist.tile([P, KB, NT // 2], BF16)
        w_sb = [None] + [persist.tile([P, KB, NT], BF16, name=f"w{j}")
                         for j in range(1, OT)]
        w_pieces = {0: [(w0a, 0, NT // 2), (w0b, NT // 2, NT // 2)]}
        for j in range(1, OT):
            w_pieces[j] = [(w_sb[j], 0, NT)]
        # fp8 W tiles (k-chunks 6..7): Q tiles DMA'd from the host, K/V tiles
        # written on device by the delta adds.
        w8_sb = [persist.tile([P, KF, NT], FP8, name=f"w8_{j}")
                 for j in range(OT)]
        wb67_sb = persist.tile([P, KF, 2 * O], BF16)  # 8x-scaled K/V base rows
        bias_sb = persist.tile([P, O3], BF16)
        a_sb = persist.tile([P, RO, I], FP8)
        bT_sb = persist.tile([P, RO, O], BF16)
        d_sb = persist.tile([P, 2, RO], F32)
        b_bc = persist.tile([P, 2, O], BF16)
        bbT = [persist.tile([P, RO, O], FP8, name=f"bbT{k}") for k in range(2)]
        bbt_tmp = persist.tile([P, RO, O], BF16)
        warm_sb = persist.tile([P, NT], BF16)

        # ---- PE pre-warm: zero a dummy tile, then issue matmuls on it so
        # the PE p-state ramp (full clock only after ~5.5us of sustained
        # busy in the cost model) burns off while the first input DMAs are
        # in flight; the real matmuls then start at full rate.
        nc.vector.memset(warm_sb[:], 0.0)
        warm_ps = psum_pool.tile([P, NT], F32, tag="ps", name="warm_ps")
        for _ in range(8):
            nc.tensor.matmul(warm_ps[:], warm_sb[:, 0:P], warm_sb[:],
                             start=True, stop=True)

        # ---- input DMAs, ordered for earliest PE start on the shared DMA
        # resource: x-chunk0 + W-tile0 pieces first, the rest of x/W next,
        # vera tensors behind them, K/V weight rows last (needed latest).
        def dma_x(c):
            cst, clen = XCHUNKS[c]
            nc.sync.dma_start(x_sb[c][:], xT_r[:, :, cst:cst + clen])

        dma_x(0)
        nc.sync.dma_start(w0a[:], wT_r[:, :, 0:NT // 2])
        nc.sync.dma_start(x8_sb[0][:], x8_r[:, :, 0:512])
        nc.sync.dma_start(w8_sb[0][:], w8q_r[:, :, 0:NT])
        dma_x(1)
        nc.sync.dma_start(w0b[:], wT_r[:, :, NT // 2:NT])
        nc.sync.dma_start(x8_sb[1][:], x8_r[:, :, 512:T])
        dma_x(2)
        nc.sync.dma_start(bias_sb[:], bias_d.ap().partition_broadcast(P))
        dma_x(3)
        dma_x(4)
        nc.sync.dma_start(d_sb[:], d_d.ap().rearrange("k (ro p) -> p k ro", p=P))
        nc.sync.dma_start(bT_sb[:], bT_d.ap().rearrange("(ro p) o -> p ro o", p=P))
        nc.sync.dma_start(b_bc[:], b_d.ap().partition_broadcast(P))
        nc.sync.dma_start(a_sb[:], a_d.ap().rearrange("(ro p) i -> p ro i", p=P))
        nc.sync.dma_start(w_sb[1][:], wT_r[:, :, NT:2 * NT])
        nc.sync.dma_start(w8_sb[1][:], w8q_r[:, :, NT:2 * NT])
        nc.sync.dma_start(wb67_sb[:], wb67_r[:])
        for j in range(2, OT):
            nc.sync.dma_start(w_sb[j][:], wT_r[:, :, j * NT:(j + 1) * NT])

        def mm_group(ps, ot, tt, wt, coff, cw, bf16_first=False):
            """The accumulation group for psum region [coff, coff+cw): one
            fp8 DoubleRow matmul (k-chunks 6..7) + KB bf16 matmuls on the
            bf16 W piece `wt` (whose columns span that same region).  The
            very first tile runs bf16-first because those operands land
            before the fp8 ones on the DMA resource."""
            ci, tj = tt_map[tt]
            lhs_x = x_sb[ci]
            if not bf16_first:
                nc.tensor.matmul(
                    ps[:, coff:coff + cw],
                    x8_slab(tt),
                    w8_sb[ot][:, :, coff:coff + cw],
                    start=True, stop=False, perf_mode=DR)
            for k in range(KB):
                nc.tensor.matmul(
                    ps[:, coff:coff + cw],
                    lhs_x[:, k, tj * P:(tj + 1) * P],
                    wt[:, k, 0:cw],
                    start=(bf16_first and k == 0),
                    stop=(not bf16_first and k == KB - 1))
            if bf16_first:
                nc.tensor.matmul(
                    ps[:, coff:coff + cw],
                    x8_slab(tt),
                    w8_sb[ot][:, :, coff:coff + cw],
                    start=False, stop=True, perf_mode=DR)

        def main_tile(ot, tt):
            """One (o-tile, token-tile): fp8 DR + bf16 accumulating matmuls
            per column piece, bias add (fp32 psum -> bf16), DMA y out."""
            if ot == OT - 1 and tt == TT - 1:
                # final tile: two independent psum tiles so the first half's
                # evict + DMA drain during the second half's matmuls, halving
                # the kernel-end chain after the very last matmul.
                SPLITS = [(0, 256), (256, 256)]
                for c, (o_off, h) in enumerate(SPLITS):
                    psh = psum_pool.tile([P, h], F32, tag="ps",
                                         name=f"ps_last_{c}")
                    nc.tensor.matmul(
                        psh[:],
                        x8_slab(tt),
                        w8_sb[ot][:, :, o_off:o_off + h],
                        start=True, stop=False, perf_mode=DR)
                    ci, tj = tt_map[tt]
                    (wt, _, _), = w_pieces[ot]
                    for k in range(KB):
                        nc.tensor.matmul(
                            psh[:],
                            x_sb[ci][:, k, tj * P:(tj + 1) * P],
                            wt[:, k, o_off:o_off + h],
                            start=False, stop=(k == KB - 1))
                    o0 = ot * NT + o_off
                    ysh = ypool.tile([P, h], BF16, tag="ys", name=f"ys_last_{c}")
                    nc.vector.tensor_tensor(
                        ysh[:], psh[:], bias_sb[:, o0:o0 + h], ADD)
                    # first half's write-back goes out on the idle ACT queue
                    # so the final (critical) dispatch isn't serialized
                    # behind it on SP
                    eng = nc.scalar if c == 0 else nc.sync
                    eng.dma_start(
                        y_d.ap()[tt * P:(tt + 1) * P, o0:o0 + h], ysh[:])
                return
            ps = psum_pool.tile([P, NT], F32, tag="ps", name=f"ps_{ot}_{tt}")
            ys = ypool.tile([P, NT], BF16, tag="ys", name=f"ys_{ot}_{tt}")
            for (wt, coff, cw) in w_pieces[ot]:
                mm_group(ps, ot, tt, wt, coff, cw,
                         bf16_first=(ot == 0 and tt == 0))
            nc.vector.tensor_tensor(
                ys[:], ps[:], bias_sb[:, ot * NT:(ot + 1) * NT], ADD)
            nc.sync.dma_start(
                y_d.ap()[tt * P:(tt + 1) * P, ot * NT:(ot + 1) * NT], ys[:])

        def delta_group(k, ic, oth):
            """delta.T[i-block ic, o-half oth] for projection k, accumulated
            over both rank chunks, then folded into the K/V weights: bf16
            chunks add in place; fp8 chunks (ic 6..7, 8x-scaled via the host-
            scaled vera_A columns) combine with the 8x-scaled bf16 base rows
            and write the fp8 tile in one rounding."""
            pd = psum_pool.tile([P, NT], F32, tag="ps", name=f"pd_{k}_{ic}_{oth}")
            nc.tensor.matmul(
                pd[:],
                a_sb[:, :, ic * P:(ic + 1) * P],
                bbT[k][:, :, oth * NT:(oth + 1) * NT],
                start=True, stop=True, perf_mode=DR)
            j = 2 + 2 * k + oth
            if ic < KB:
                wj = w_sb[j]
                nc.gpsimd.scalar_tensor_tensor(
                    wj[:, ic, :], pd[:], 1.0 / 8192.0, wj[:, ic, :], MUL, ADD)
            else:
                icf = ic - KB
                boff = k * O + oth * NT
                nc.gpsimd.scalar_tensor_tensor(
                    w8_sb[j][:, icf, :], pd[:], SWX / 8192.0,
                    wb67_sb[:, icf, boff:boff + NT], MUL, ADD)

        def emit_bbT():
            # bbT_k[r, o] = d[k, r] * b[k, o] * B.T[r, o]  (DVE, bf16 out)
            for k in range(2):
                nc.vector.tensor_tensor(
                    bbt_tmp[:], bT_sb[:],
                    d_sb[:, k, :, None].to_broadcast([P, RO, O]), MUL)
                nc.vector.tensor_tensor(
                    bbT[k][:], bbt_tmp[:],
                    b_bc[:, k, None, :].to_broadcast([P, RO, O]), MUL)

        # ---- phase ot0 (Q first half): PE warms up while the rest of the
        # inputs stream in.  bbT emitted after tt3 so the DVE queue first
        # drains the early psum evictions.
        for tt in range(TT):
            main_tile(0, tt)
            if tt == 1:
                emit_bbT()

        # ---- phase ot1 (Q second half) with the 32 delta groups woven in,
        # two per token tile: PE alternates 2 delta MM pairs + main MMs;
        # DVE drains one delta psum per ~2us of PE work, so no stalls.
        kgroups = [(0, ic, oth) for ic in range(KO) for oth in range(2)]
        vgroups = [(1, ic, oth) for ic in range(KO) for oth in range(2)]
        for tt in range(TT):
            main_tile(1, tt)
            delta_group(*kgroups[tt])

        # ---- phases ot2..ot5 (K then V, patched with the vera delta; the V
        # delta groups are woven into the ot2 phase, before ot4/ot5 read them)
        for tt in range(TT):
            delta_group(*vgroups[tt])
            main_tile(2, tt)
        for ot in range(3, OT):
            for tt in range(TT):
                main_tile(ot, tt)


_cached_nc = None


def _get_nc():
    global _cached_nc
    if _cached_nc is None:
        _cached_nc = _build_kernel()
    return _cached_nc


def _make_in_maps(x, base_weight, base_bias, vera_A, vera_B, vera_d, vera_b):
    x2 = np.asarray(x, dtype=np.float32).reshape(T_TOTAL, I)
    Wb = np.asarray(base_weight, dtype=np.float32)
    wT = np.ascontiguousarray(Wb[:, :IB].T.astype(BF))
    # fp8 side: Q rows as fp8(8*W); K/V rows as bf16(8*W) (the device adds the
    # 8x-scaled delta and performs the single fp8 rounding itself)
    w8q = np.ascontiguousarray((Wb[:O, IB:] * SWX).T.astype(F8))
    wb67 = np.ascontiguousarray(
        (Wb[O:, IB:].astype(BF).astype(np.float32).T * SWX).astype(BF))
    bT = np.ascontiguousarray(np.asarray(vera_B, dtype=np.float32).T.astype(BF))
    bias = np.ascontiguousarray(np.asarray(base_bias, dtype=np.float32).astype(BF))
    # fp8 delta operands: 32*A and (via vera_d) 256*bbT keep e4m3 in range;
    # the device descales the 8192x psum on the ACT engine before the add
    a = np.ascontiguousarray((np.asarray(vera_A, dtype=np.float32) * 32).astype(F8))
    d = np.ascontiguousarray(np.asarray(vera_d, dtype=np.float32) * 256)
    b = np.ascontiguousarray(np.asarray(vera_b, dtype=np.float32).astype(BF))
    in_maps = []
    for c in range(N_CORES):
        xs = x2[c * T:(c + 1) * T]
        xT_c = np.ascontiguousarray(xs[:, :IB].T.astype(BF))
        x8_c = np.ascontiguousarray((xs[:, IB:] / SWX).T.astype(F8))
        in_maps.append({
            "xT": xT_c, "x8T": x8_c, "wT": wT, "w8q": w8q, "wb67": wb67,
            "bias": bias, "vera_A": a, "vera_BT": bT, "vera_d": d, "vera_b": b,
        })
    return in_maps


def _run_coresim(nc, in_maps):
    """Fallback: interpret the BIR per core (bit-accurate, no hardware)."""
    from concourse.bass_interp import CoreSim

    shards = []
    for in_map in in_maps:
        sim = CoreSim(nc, trace=False)
        for name, val in in_map.items():
            sim.tensor(name)[:] = val
        sim.simulate(check_with_hw=False)
        shards.append(np.array(sim.tensor("y")))
    return shards


def kernel(x, base_weight, base_bias, vera_A, vera_B, vera_d, vera_b):
    nc = _get_nc()
    in_maps = _make_in_maps(x, base_weight, base_bias, vera_A, vera_B,
                            vera_d, vera_b)
    try:
        res = bass_utils.run_bass_kernel_spmd(nc, in_maps,
                                              core_ids=list(range(N_CORES)))
        shards = [np.asarray(res.results[c]["y"]) for c in range(N_CORES)]
    except Exception:
        # The axon PJRT execute path can be unavailable in some containers;
        # fall back to interpreting the same BIR so results stay correct.
        shards = _run_coresim(nc, in_maps)
    y = np.concatenate(shards, axis=0).astype(np.float32)
    return y.reshape(B, S, O3)
